# revision 10
# baseline (speedup 1.0000x reference)
"""MediatorAttention Trainium2 kernel.

Problem (reference.py): B=4, L=4096, D=1024, H=16 heads, d=64, M=64 mediators.
  q = x @ Wq;  k,v = split(x @ Wkv)
  stage1: mk_v[b,h] = softmax(med_h @ k_bh^T / 8, axis=L) @ v_bh      [M, d]
  stage2: out_bh   = softmax(q_bh @ med_h^T / 8, axis=M) @ mk_v[b,h]  [L, d]
  y = concat_heads(out) @ Wo + bo

Sharding over 8 NeuronCores: core c handles batch b = c//2 and head-half
hh = c%2 (8 heads). Attention is independent per (batch, head), so stage-1's
softmax over L stays core-local; no collectives. Each core produces the
partial y^T through its 8 heads' rows of Wo; the host sums the two partials
per batch, transposes, and adds bo.

Per-core dataflow (activations kept "transposed": features on SBUF
partitions, sequence rows on the free axis — no on-chip transposes needed;
the host passes x[b]^T):
  P1  per 512-row chunk (one resident xT chunk tile, streamed once):
        qT,kT = W_tile^T @ xT   (W stationary)     v = x @ Wv (xT stationary)
  P3  per 128-row tile: l1 = k_tile @ medT (natural), E1 = exp(l1/8)
      u[M, d|s] += E1^T @ [v | 1]  (ones column gives the softmax denom s)
      mkv = u / s
  P4  per chunk: E2T = exp(medT_blk^T @ qT / 8)  [M, rows]
      z = zpat^T @ E2T (per-head row sums, batched into one [8,512] psum),
      zb = sel2^T @ (1/z)  (row-broadcast via K=2 matmul), E2n = E2T * zb,
      attT = mkv_blk^T @ E2n
  P5  yT += Wo_tile^T @ attT

Head-pair batching: two heads (2p, 2p+1) share each 128-partition tile with
block-diagonal medT / mkv stationaries so all matmuls use full K=128.
"""

import numpy as np
import ml_dtypes

import concourse.bass as bass
import concourse.tile as tile
from concourse import bacc, mybir

BF16 = mybir.dt.bfloat16
F32 = mybir.dt.float32
F32R = mybir.dt.float32r

B, L, D, H, M = 4, 4096, 1024, 16, 64
d = D // H          # 64
NCORES = 8
HVAL = 512          # head-half width: 8 heads * 64
NPAIR = 4           # head pairs per core
KT = D // 128       # 8 feature k-tiles
SCALE = 0.125       # d ** -0.5


def build_module(Lr=L, repeat=1):
    """Build the single-core Bass module (SPMD-replicated across 8 cores).

    repeat>1 re-executes the whole pipeline (timing amplification only).
    """
    nchunk = Lr // 512
    nrt = Lr // 128

    nc = bacc.Bacc("TRN2", target_bir_lowering=False, debug=False,
                   num_devices=NCORES)

    xT = nc.dram_tensor("xT", [D, Lr], BF16, kind="ExternalInput").ap()
    wq = nc.dram_tensor("wq", [D, HVAL], BF16, kind="ExternalInput").ap()
    wk = nc.dram_tensor("wk", [D, HVAL], BF16, kind="ExternalInput").ap()
    wv = nc.dram_tensor("wv", [D, HVAL], BF16, kind="ExternalInput").ap()
    wo = nc.dram_tensor("wo", [HVAL, D], BF16, kind="ExternalInput").ap()
    medblk = nc.dram_tensor("medblk", [NPAIR, 128, 128], BF16,
                            kind="ExternalInput").ap()
    zpat = nc.dram_tensor("zpat", [128, 2], BF16, kind="ExternalInput").ap()
    sel2 = nc.dram_tensor("sel2", [2, 128], F32, kind="ExternalInput").ap()
    yT = nc.dram_tensor("yT", [D, Lr], F32, kind="ExternalOutput").ap()

    xTr = xT.rearrange("(a b) c -> b a c", a=KT)      # [128, KT, Lr]

    with tile.TileContext(nc) as tc:
        with tc.tile_pool(name="consts", bufs=1) as consts, \
             tc.tile_pool(name="acts", bufs=1) as acts:
            # --- constants / weights resident in SBUF (SWDGE queue so they
            # don't serialize ahead of the xT streaming loads on HWDGE) ---
            wq_sb = consts.tile([128, KT, HVAL], BF16)
            wk_sb = consts.tile([128, KT, HVAL], BF16)
            wv_sb = consts.tile([128, KT, HVAL], BF16)
            wo_sb = consts.tile([128, NPAIR, D], BF16)
            med_sb = consts.tile([128, NPAIR, 128], BF16)
            zpat_sb = consts.tile([128, 2], BF16)
            sel2_sb = consts.tile([2, 128], F32)
            nc.gpsimd.dma_start(out=wq_sb[:], in_=wq.rearrange("(a b) c -> b a c", a=KT))
            nc.gpsimd.dma_start(out=wk_sb[:], in_=wk.rearrange("(a b) c -> b a c", a=KT))
            nc.gpsimd.dma_start(out=wv_sb[:], in_=wv.rearrange("(a b) c -> b a c", a=KT))
            nc.gpsimd.dma_start(out=wo_sb[:], in_=wo.rearrange("(a b) c -> b a c", a=NPAIR))
            nc.gpsimd.dma_start(out=med_sb[:], in_=medblk.rearrange("a b c -> b a c"))
            nc.gpsimd.dma_start(out=zpat_sb[:], in_=zpat[:])
            nc.gpsimd.dma_start(out=sel2_sb[:], in_=sel2[:])

            # --- persistent activations ---
            qT_sb = acts.tile([128, NPAIR, Lr], BF16)
            kT_sb = acts.tile([128, NPAIR, Lr], BF16)
            v_sb = acts.tile([128, nrt, NPAIR, 129], BF16)
            attT_sb = acts.tile([128, NPAIR, Lr], BF16)
            mkv_sb = acts.tile([128, NPAIR, 128], BF16)

            for _rep in range(repeat):
                # ========== P1: q/k/v projections (single xT stream) ==========
                with tc.tile_pool(name="p1x", bufs=2) as p1x, \
                     tc.tile_pool(name="p1ps", bufs=8, space="PSUM") as p1ps:
                    for ch in range(nchunk):
                        csl = slice(ch * 512, (ch + 1) * 512)
                        xt = p1x.tile([128, KT, 512], BF16)
                        nc.sync.dma_start(out=xt[:], in_=xTr[:, :, csl])
                        for half, (dst, w_sb) in enumerate(
                                ((qT_sb, wq_sb), (kT_sb, wk_sb))):
                            ps = [p1ps.tile([128, 512], F32, tag="pps",
                                            name=f"ps{half}{i}") for i in range(4)]
                            for kt in range(KT):
                                for p in range(NPAIR):
                                    nc.tensor.matmul(
                                        ps[p][:], w_sb[:, kt, p * 128:(p + 1) * 128],
                                        xt[:, kt, :],
                                        start=(kt == 0), stop=(kt == KT - 1))
                            for p in range(NPAIR):
                                nc.vector.tensor_copy(dst[:, p, csl], ps[p][:])
                        psv = [p1ps.tile([128, 512], F32, tag="pps",
                                         name=f"psv{i}") for i in range(4)]
                        for kt in range(KT):
                            for rb in range(4):
                                nc.tensor.matmul(psv[rb][:],
                                                 xt[:, kt, rb * 128:(rb + 1) * 128],
                                                 wv_sb[:, kt, :],
                                                 start=(kt == 0), stop=(kt == KT - 1))
                        for rb in range(4):
                            rt = ch * 4 + rb
                            nc.vector.tensor_copy(
                                v_sb[:, rt, :, 0:128],
                                psv[rb][:].rearrange("p (a b) -> p a b", a=NPAIR))
                            nc.vector.memset(v_sb[:, rt, :, 128:129], 1.0)

                # ================= P3: stage 1 =================
                with tc.tile_pool(name="s1l", bufs=2, space="PSUM") as s1l, \
                     tc.tile_pool(name="s1u", bufs=1, space="PSUM") as s1u, \
                     tc.tile_pool(name="s1e", bufs=3) as s1e, \
                     tc.tile_pool(name="s1s", bufs=4) as s1s:
                    ups = [s1u.tile([128, 129], F32, tag=f"u{p}", name=f"u{p}")
                           for p in range(NPAIR)]
                    for rt in range(nrt):
                        l1 = s1l.tile([128, 512], F32)
                        for p in range(NPAIR):
                            nc.tensor.matmul(l1[:, p * 128:(p + 1) * 128],
                                             kT_sb[:, p, rt * 128:(rt + 1) * 128],
                                             med_sb[:, p, :], start=True, stop=True)
                        e1 = s1e.tile([128, 512], BF16)
                        nc.scalar.activation(e1[:], l1[:],
                                             mybir.ActivationFunctionType.Exp,
                                             scale=SCALE)
                        for p in range(NPAIR):
                            nc.tensor.matmul(ups[p][:], e1[:, p * 128:(p + 1) * 128],
                                             v_sb[:, rt, p, :],
                                             start=(rt == 0), stop=(rt == nrt - 1))
                    for p in range(NPAIR):
                        srecip = s1s.tile([128, 1], F32)
                        nc.vector.reciprocal(srecip[:], ups[p][:, 128:129])
                        nc.vector.memset(mkv_sb[:, p, :], 0.0)
                        for hi in range(2):
                            s_ = slice(hi * 64, (hi + 1) * 64)
                            nc.vector.tensor_scalar_mul(mkv_sb[s_, p, s_],
                                                        ups[p][s_, s_], srecip[s_])

                # ================= P4: stage 2 =================
                with tc.tile_pool(name="s2l", bufs=2, space="PSUM") as s2l, \
                     tc.tile_pool(name="s2z", bufs=2, space="PSUM") as s2z, \
                     tc.tile_pool(name="s2b", bufs=2, space="PSUM") as s2b, \
                     tc.tile_pool(name="s2a", bufs=2, space="PSUM") as s2a, \
                     tc.tile_pool(name="s2e", bufs=8) as s2e, \
                     tc.tile_pool(name="s2n", bufs=3) as s2n, \
                     tc.tile_pool(name="s2r", bufs=8) as s2r:
                    for ch in range(nchunk):
                        csl = slice(ch * 512, (ch + 1) * 512)
                        e2s = []
                        zrs = []
                        for p in range(NPAIR):
                            l2 = s2l.tile([128, 512], F32, tag="l2")
                            nc.tensor.matmul(l2[:], med_sb[:, p, :], qT_sb[:, p, csl],
                                             start=True, stop=True)
                            e2 = s2e.tile([128, 512], BF16, tag="e2", name=f"e2_{p}")
                            nc.scalar.activation(e2[:], l2[:],
                                                 mybir.ActivationFunctionType.Exp,
                                                 scale=SCALE)
                            zp = s2z.tile([2, 512], F32, tag="z", name=f"z_{p}")
                            nc.tensor.matmul(zp[:], zpat_sb[:], e2[:],
                                             start=True, stop=True)
                            zr = s2r.tile([2, 512], F32, tag="zr", name=f"zr_{p}")
                            nc.vector.reciprocal_approx_fast(out=zr[:], in_=zp[:])
                            e2s.append(e2)
                            zrs.append(zr)
                        for p in range(NPAIR):
                            zb = s2b.tile([128, 512], F32, tag="zb")
                            nc.tensor.matmul(zb[:], sel2_sb[:], zrs[p][:],
                                             start=True, stop=True)
                            e2n = s2n.tile([128, 512], BF16, tag="e2n")
                            with nc.allow_low_precision(reason="softmax wt in bf16"):
                                nc.vector.tensor_mul(e2n[:], e2s[p][:], zb[:])
                            att = s2a.tile([128, 512], F32, tag="att")
                            nc.tensor.matmul(att[:], mkv_sb[:, p, :], e2n[:],
                                             start=True, stop=True)
                            nc.vector.tensor_copy(attT_sb[:, p, csl], att[:])

                # ================= P5: yT = Wo_slice^T @ attT =================
                with tc.tile_pool(name="p5ps", bufs=4, space="PSUM") as p5ps, \
                     tc.tile_pool(name="p5y", bufs=3) as p5y:
                    for ycb in range(8):
                        for ch in range(nchunk):
                            csl = slice(ch * 512, (ch + 1) * 512)
                            yp = p5ps.tile([128, 512], F32)
                            for p in range(NPAIR):
                                nc.tensor.matmul(yp[:],
                                                 wo_sb[:, p, ycb * 128:(ycb + 1) * 128],
                                                 attT_sb[:, p, csl],
                                                 start=(p == 0), stop=(p == NPAIR - 1))
                            ys = p5y.tile([128, 512], F32)
                            nc.vector.tensor_copy(ys[:], yp[:])
                            nc.sync.dma_start(out=yT[ycb * 128:(ycb + 1) * 128, csl],
                                              in_=ys[:])

    nc.compile()
    return nc


def host_prep(x, Wq, Wkv, Wo, mediator, Lr=L):
    """Slice/cast/transpose full inputs into the 8 per-core input maps."""
    bf = ml_dtypes.bfloat16
    med = np.asarray(mediator[0])  # [M, D]
    zpat = np.zeros((128, 2), np.float32)
    zpat[0:64, 0] = 1.0
    zpat[64:128, 1] = 1.0
    sel2 = np.zeros((2, 128), np.float32)
    sel2[0, 0:64] = 1.0
    sel2[1, 64:128] = 1.0

    xTb = [np.ascontiguousarray(np.asarray(x[b, :Lr, :]).T).astype(bf)
           for b in range(B)]
    in_maps = []
    for c in range(NCORES):
        b, hh = c // 2, c % 2
        cs = slice(hh * HVAL, (hh + 1) * HVAL)
        blk = np.zeros((NPAIR, 128, 128), np.float32)
        for p in range(NPAIR):
            for hi in range(2):
                h = hh * 8 + 2 * p + hi
                s_ = slice(hi * 64, (hi + 1) * 64)
                blk[p][s_, s_] = med[:, h * 64:(h + 1) * 64].T  # medT_h [d, M]
        in_maps.append({
            "xT": xTb[b],
            "wq": np.asarray(Wq[:, cs]).astype(bf),
            "wk": np.asarray(Wkv[:, cs]).astype(bf),
            "wv": np.asarray(Wkv[:, D + hh * HVAL:D + (hh + 1) * HVAL]).astype(bf),
            "wo": np.asarray(Wo[cs, :]).astype(bf),
            "medblk": blk.astype(bf),
            "zpat": zpat.astype(bf),
            "sel2": sel2,
        })
    return in_maps


def host_combine(results, bo, Lr=L):
    y = np.empty((B, Lr, D), np.float32)
    bo = np.asarray(bo, np.float32)
    for b in range(B):
        yt = results[2 * b]["yT"] + results[2 * b + 1]["yT"]
        y[b] = yt.T + bo
    return y


_NC_CACHE = {}


def kernel(x, Wq, Wkv, Wo, bo, mediator):
    from concourse.bass_utils import run_bass_kernel_spmd
    if "nc" not in _NC_CACHE:
        _NC_CACHE["nc"] = build_module(L)
    nc = _NC_CACHE["nc"]
    in_maps = host_prep(x, Wq, Wkv, Wo, mediator, L)
    res = run_bass_kernel_spmd(nc, in_maps, list(range(NCORES)))
    return host_combine(res.results, bo, L)


# revision 16
# speedup vs baseline: 1.3332x; 1.3332x over previous
"""MediatorAttention Trainium2 kernel.

Problem (reference.py): B=4, L=4096, D=1024, H=16 heads, d=64, M=64 mediators.
  q = x @ Wq;  k,v = split(x @ Wkv)
  stage1: mk_v[b,h] = softmax(med_h @ k_bh^T / 8, axis=L) @ v_bh      [M, d]
  stage2: out_bh   = softmax(q_bh @ med_h^T / 8, axis=M) @ mk_v[b,h]  [L, d]
  y = concat_heads(out) @ Wo + bo

Sharding over 8 NeuronCores: core c handles batch b = c//2 and head-half
hh = c%2 (8 heads). Attention is independent per (batch, head), so stage-1's
softmax over L stays core-local; no collectives. Each core produces the
partial y^T through its 8 heads' rows of Wo; the host sums the two partials
per batch, transposes, and adds bo.

Per-core dataflow (activations kept "transposed": features on SBUF
partitions, sequence rows on the free axis — no on-chip transposes needed;
the host passes x[b]^T):
  P1  per 512-row chunk (one resident xT chunk tile, streamed once):
        qT,kT = W_tile^T @ xT   (W stationary)     v = x @ Wv (xT stationary)
  P3  per 128-row tile: l1 = k_tile @ medT (natural), E1 = exp(l1/8)
      u[M, d|s] += E1^T @ [v | 1]  (ones column gives the softmax denom s)
      mkv = u / s
  P4  per chunk: E2T = exp(medT_blk^T @ qT / 8)  [M, rows]
      z = zpat^T @ E2T (per-head row sums, batched into one [8,512] psum),
      zb = sel2^T @ (1/z)  (row-broadcast via K=2 matmul), E2n = E2T * zb,
      attT = mkv_blk^T @ E2n
  P5  yT += Wo_tile^T @ attT

Head-pair batching: two heads (2p, 2p+1) share each 128-partition tile with
block-diagonal medT / mkv stationaries so all matmuls use full K=128.
"""

import numpy as np
import ml_dtypes

import concourse.bass as bass
import concourse.tile as tile
from concourse import bacc, mybir

BF16 = mybir.dt.bfloat16
F32 = mybir.dt.float32
F32R = mybir.dt.float32r

B, L, D, H, M = 4, 4096, 1024, 16, 64
d = D // H          # 64
NCORES = 8
HVAL = 512          # head-half width: 8 heads * 64
NPAIR = 4           # head pairs per core
KT = D // 128       # 8 feature k-tiles
SCALE = 0.125       # d ** -0.5


def build_module(Lr=L, repeat=1):
    """Build the single-core Bass module (SPMD-replicated across 8 cores).

    repeat>1 re-executes the whole pipeline (timing amplification only).
    """
    nchunk = Lr // 512
    nrt = Lr // 128

    nc = bacc.Bacc("TRN2", target_bir_lowering=False, debug=False,
                   num_devices=NCORES)

    xT = nc.dram_tensor("xT", [D, Lr], BF16, kind="ExternalInput").ap()
    wq = nc.dram_tensor("wq", [D, HVAL], BF16, kind="ExternalInput").ap()
    wk = nc.dram_tensor("wk", [D, HVAL], BF16, kind="ExternalInput").ap()
    wv = nc.dram_tensor("wv", [D, HVAL], BF16, kind="ExternalInput").ap()
    wo = nc.dram_tensor("wo", [HVAL, D], BF16, kind="ExternalInput").ap()
    medblk = nc.dram_tensor("medblk", [NPAIR, 128, 128], BF16,
                            kind="ExternalInput").ap()
    zpat = nc.dram_tensor("zpat", [128, 2], BF16, kind="ExternalInput").ap()
    sel2 = nc.dram_tensor("sel2", [2, 128], F32, kind="ExternalInput").ap()
    yT = nc.dram_tensor("yT", [D, Lr], F32, kind="ExternalOutput").ap()

    xTr = xT.rearrange("(a b) c -> b a c", a=KT)      # [128, KT, Lr]

    with tile.TileContext(nc) as tc:
        with tc.tile_pool(name="consts", bufs=1) as consts, \
             tc.tile_pool(name="acts", bufs=1) as acts:
            # --- constants / weights resident in SBUF (SWDGE queue so they
            # don't serialize ahead of the xT streaming loads on HWDGE) ---
            wq_sb = consts.tile([128, KT, HVAL], BF16)
            wk_sb = consts.tile([128, KT, HVAL], BF16)
            wv_sb = consts.tile([128, KT, HVAL], BF16)
            wo_sb = consts.tile([128, NPAIR, D], BF16)
            med_sb = consts.tile([128, NPAIR, 128], BF16)
            zpat_sb = consts.tile([128, 2], BF16)
            sel2_sb = consts.tile([2, 128], F32)
            sel2_r = consts.tile([2, 128], F32R)
            nc.gpsimd.dma_start(out=wq_sb[:], in_=wq.rearrange("(a b) c -> b a c", a=KT))
            nc.gpsimd.dma_start(out=wk_sb[:], in_=wk.rearrange("(a b) c -> b a c", a=KT))
            nc.gpsimd.dma_start(out=wv_sb[:], in_=wv.rearrange("(a b) c -> b a c", a=KT))
            nc.gpsimd.dma_start(out=wo_sb[:], in_=wo.rearrange("(a b) c -> b a c", a=NPAIR))
            nc.gpsimd.dma_start(out=med_sb[:], in_=medblk.rearrange("a b c -> b a c"))
            nc.gpsimd.dma_start(out=zpat_sb[:], in_=zpat[:])
            nc.gpsimd.dma_start(out=sel2_sb[:], in_=sel2[:])

            # --- persistent activations ---
            qT_sb = acts.tile([128, NPAIR, Lr], BF16)
            kT_sb = acts.tile([128, NPAIR, Lr], BF16)
            v_sb = acts.tile([128, nrt, NPAIR, 129], BF16)
            mkv_sb = acts.tile([128, NPAIR, 128], BF16)

            nc.vector.memset(v_sb[:, :, :, 128:129], 1.0)
            nc.vector.tensor_copy(sel2_r[:], sel2_sb[:])

            for _rep in range(repeat):
                # ========== P1: q/k/v projections (single xT stream) ==========
                with tc.tile_pool(name="p1x", bufs=2) as p1x, \
                     tc.tile_pool(name="p1ps", bufs=8, space="PSUM") as p1ps:
                    for ch in range(nchunk):
                        csl = slice(ch * 512, (ch + 1) * 512)
                        xt = p1x.tile([128, KT, 512], BF16)
                        nc.sync.dma_start(out=xt[:], in_=xTr[:, :, csl])
                        for half, (dst, w_sb) in enumerate(
                                ((qT_sb, wq_sb), (kT_sb, wk_sb))):
                            ps = [p1ps.tile([128, 512], F32, tag="pps",
                                            name=f"ps{half}{i}") for i in range(4)]
                            for kt in range(KT):
                                for p in range(NPAIR):
                                    nc.tensor.matmul(
                                        ps[p][:], w_sb[:, kt, p * 128:(p + 1) * 128],
                                        xt[:, kt, :],
                                        start=(kt == 0), stop=(kt == KT - 1))
                            for p in range(NPAIR):
                                nc.vector.tensor_copy(dst[:, p, csl], ps[p][:])
                        psv = [p1ps.tile([128, 512], F32, tag="pps",
                                         name=f"psv{i}") for i in range(4)]
                        for kt in range(KT):
                            for rb in range(4):
                                nc.tensor.matmul(psv[rb][:],
                                                 xt[:, kt, rb * 128:(rb + 1) * 128],
                                                 wv_sb[:, kt, :],
                                                 start=(kt == 0), stop=(kt == KT - 1))
                        for rb in range(4):
                            rt = ch * 4 + rb
                            nc.vector.tensor_copy(
                                v_sb[:, rt, :, 0:128],
                                psv[rb][:].rearrange("p (a b) -> p a b", a=NPAIR))

                # ================= P3: stage 1 =================
                with tc.tile_pool(name="s1l", bufs=3, space="PSUM") as s1l, \
                     tc.tile_pool(name="s1u", bufs=1, space="PSUM") as s1u, \
                     tc.tile_pool(name="s1e", bufs=3) as s1e, \
                     tc.tile_pool(name="s1s", bufs=4) as s1s:
                    ups = [s1u.tile([128, 129], F32, tag=f"u{p}", name=f"u{p}")
                           for p in range(NPAIR)]
                    for rt in range(nrt):
                        l1 = s1l.tile([128, 512], F32)
                        for p in range(NPAIR):
                            nc.tensor.matmul(l1[:, p * 128:(p + 1) * 128],
                                             kT_sb[:, p, rt * 128:(rt + 1) * 128],
                                             med_sb[:, p, :], start=True, stop=True)
                        e1 = s1e.tile([128, 512], BF16)
                        nc.scalar.activation(e1[:], l1[:],
                                             mybir.ActivationFunctionType.Exp,
                                             scale=SCALE)
                        for p in range(NPAIR):
                            nc.tensor.matmul(ups[p][:], e1[:, p * 128:(p + 1) * 128],
                                             v_sb[:, rt, p, :],
                                             start=(rt == 0), stop=(rt == nrt - 1))
                    for p in range(NPAIR):
                        srecip = s1s.tile([128, 1], F32)
                        nc.vector.reciprocal(srecip[:], ups[p][:, 128:129])
                        nc.vector.memset(mkv_sb[:, p, :], 0.0)
                        for hi in range(2):
                            s_ = slice(hi * 64, (hi + 1) * 64)
                            nc.vector.tensor_scalar_mul(mkv_sb[s_, p, s_],
                                                        ups[p][s_, s_], srecip[s_])

                # ========== P4+P5 fused: stage 2 + output projection ==========
                # Per chunk: compute normalized attT for all 4 pairs, then
                # immediately run the 8 Wo column-blocks for that chunk so
                # P5's matmuls fill PE while P4's ACT/DVE chain drains.
                with tc.tile_pool(name="s2l", bufs=1, space="PSUM") as s2l, \
                     tc.tile_pool(name="s2z", bufs=1, space="PSUM") as s2z, \
                     tc.tile_pool(name="s2b", bufs=1, space="PSUM") as s2b, \
                     tc.tile_pool(name="s2a", bufs=2, space="PSUM") as s2a, \
                     tc.tile_pool(name="p5ps", bufs=3, space="PSUM") as p5ps, \
                     tc.tile_pool(name="s2e", bufs=8) as s2e, \
                     tc.tile_pool(name="s2n", bufs=3) as s2n, \
                     tc.tile_pool(name="s2r", bufs=8) as s2r, \
                     tc.tile_pool(name="s2t", bufs=2) as s2t, \
                     tc.tile_pool(name="p5y", bufs=4) as p5y:
                    for ch in range(nchunk):
                        csl = slice(ch * 512, (ch + 1) * 512)
                        attc = s2t.tile([128, NPAIR, 512], BF16, tag="attc")
                        e2s = []
                        zrs = []
                        for p in range(NPAIR):
                            l2 = s2l.tile([128, 512], F32, tag="l2")
                            nc.tensor.matmul(l2[:], med_sb[:, p, :], qT_sb[:, p, csl],
                                             start=True, stop=True)
                            e2 = s2e.tile([128, 512], BF16, tag="e2", name=f"e2_{p}")
                            nc.scalar.activation(e2[:], l2[:],
                                                 mybir.ActivationFunctionType.Exp,
                                                 scale=SCALE)
                            zp = s2z.tile([2, 512], F32, tag="z", name=f"z_{p}")
                            nc.tensor.matmul(zp[:], zpat_sb[:], e2[:],
                                             start=True, stop=True)
                            zr = s2r.tile([2, 512], F32, tag="zr", name=f"zr_{p}")
                            nc.vector.reciprocal_approx_fast(out=zr[:], in_=zp[:])
                            zrr = s2r.tile([2, 512], F32R, tag="zrr", name=f"zrr_{p}")
                            nc.gpsimd.tensor_copy(zrr[:], zr[:])
                            e2s.append(e2)
                            zrs.append(zrr)
                        for p in range(NPAIR):
                            zb = s2b.tile([128, 512], F32, tag="zb")
                            nc.tensor.matmul(zb[:], sel2_r[:], zrs[p][:],
                                             start=True, stop=True)
                            e2n = s2n.tile([128, 512], BF16, tag="e2n")
                            with nc.allow_low_precision(reason="softmax wt in bf16"):
                                nc.vector.tensor_mul(e2n[:], e2s[p][:], zb[:])
                            att = s2a.tile([128, 512], F32, tag="att")
                            nc.tensor.matmul(att[:], mkv_sb[:, p, :], e2n[:],
                                             start=True, stop=True)
                            nc.scalar.copy(attc[:, p, :], att[:])
                        for ycb in range(8):
                            yp = p5ps.tile([128, 512], F32)
                            for p in range(NPAIR):
                                nc.tensor.matmul(yp[:],
                                                 wo_sb[:, p, ycb * 128:(ycb + 1) * 128],
                                                 attc[:, p, :],
                                                 start=(p == 0), stop=(p == NPAIR - 1))
                            ys = p5y.tile([128, 512], F32)
                            if ycb % 2 == 0:
                                nc.vector.tensor_copy(ys[:], yp[:])
                            else:
                                nc.scalar.copy(ys[:], yp[:])
                            nc.sync.dma_start(out=yT[ycb * 128:(ycb + 1) * 128, csl],
                                              in_=ys[:])

    nc.compile()
    return nc


def host_prep(x, Wq, Wkv, Wo, mediator, Lr=L):
    """Slice/cast/transpose full inputs into the 8 per-core input maps."""
    bf = ml_dtypes.bfloat16
    med = np.asarray(mediator[0])  # [M, D]
    zpat = np.zeros((128, 2), np.float32)
    zpat[0:64, 0] = 1.0
    zpat[64:128, 1] = 1.0
    sel2 = np.zeros((2, 128), np.float32)
    sel2[0, 0:64] = 1.0
    sel2[1, 64:128] = 1.0

    xTb = [np.ascontiguousarray(np.asarray(x[b, :Lr, :]).T).astype(bf)
           for b in range(B)]
    in_maps = []
    for c in range(NCORES):
        b, hh = c // 2, c % 2
        cs = slice(hh * HVAL, (hh + 1) * HVAL)
        blk = np.zeros((NPAIR, 128, 128), np.float32)
        for p in range(NPAIR):
            for hi in range(2):
                h = hh * 8 + 2 * p + hi
                s_ = slice(hi * 64, (hi + 1) * 64)
                blk[p][s_, s_] = med[:, h * 64:(h + 1) * 64].T  # medT_h [d, M]
        in_maps.append({
            "xT": xTb[b],
            "wq": np.asarray(Wq[:, cs]).astype(bf),
            "wk": np.asarray(Wkv[:, cs]).astype(bf),
            "wv": np.asarray(Wkv[:, D + hh * HVAL:D + (hh + 1) * HVAL]).astype(bf),
            "wo": np.asarray(Wo[cs, :]).astype(bf),
            "medblk": blk.astype(bf),
            "zpat": zpat.astype(bf),
            "sel2": sel2,
        })
    return in_maps


def host_combine(results, bo, Lr=L):
    y = np.empty((B, Lr, D), np.float32)
    bo = np.asarray(bo, np.float32)
    for b in range(B):
        yt = results[2 * b]["yT"] + results[2 * b + 1]["yT"]
        y[b] = yt.T + bo
    return y


_NC_CACHE = {}


def kernel(x, Wq, Wkv, Wo, bo, mediator):
    from concourse.bass_utils import run_bass_kernel_spmd
    if "nc" not in _NC_CACHE:
        _NC_CACHE["nc"] = build_module(L)
    nc = _NC_CACHE["nc"]
    in_maps = host_prep(x, Wq, Wkv, Wo, mediator, L)
    res = run_bass_kernel_spmd(nc, in_maps, list(range(NCORES)))
    return host_combine(res.results, bo, L)
